# revision 15
# baseline (speedup 1.0000x reference)
"""Trainium2 Bass kernel for nn_DenseAttentionOneHead — collective-free.

out_b = X_b (W^T (X_b^T X_b)).  The D (=1024) output columns split into 8
independent 256-column slices (4 per batch): per core,
  S_sl = X_b^T X_b[:, sl]     ([1024, 256], full-batch contraction)
  M_sl = W^T S_sl             ([1024, 256])
  out[:, sl] = X_b M_sl       ([4096, 256])

v5: xf streams in 512KB chunk-pairs at full HBM bandwidth; every X^T
tile DMA (xtt, host pre-tiled per output row-tile) is gated behind the
arrival of xf pair 12 via a 1-column dummy copy (so the Tile scheduler
cannot hoist X^T traffic into the S window), with a 16-deep ring giving
the out phase deep lookahead.  W rides the scalar ring between the
xf odds and the M phase.  Output is fp16 (host upcasts) written on the
sync ring behind the X^T evens.  A junk-matmul warmup opens the HAM
clock gate early.
"""

import numpy as np

import concourse.mybir as mybir
import concourse.tile as tile
from concourse import bacc
from concourse.bass_utils import run_bass_kernel_spmd

F32 = mybir.dt.float32
F16 = mybir.dt.float16
P = 128
D = 1024
B = 2
N = 4096
NCORES = 8
GROUP = 4            # cores per batch
SL = D // GROUP      # 256-column slice per core
NO = D // P          # 8 contraction chunks of D
NCH = N // P         # 32 row chunks of the full batch
NT = N // P          # 32 output row tiles
NPAIR = NCH // 2     # 16 xf 512KB chunk-pairs
XRING = 16           # all 16 xf pairs resident (8MB): DMAs free-run
TRING = 16           # X^T tile ring depth (4MB)
GATE = 12            # xf pair whose arrival releases the X^T stream

_compiled = None


def _build():
    nc = bacc.Bacc(None, target_bir_lowering=False, debug=False, num_devices=NCORES)

    # xf arrives column-rotated per core (its 256 target columns first) and
    # wf row-rotated identically, so the same program computes every slice.
    # xtt[nt] is X^T pre-tiled for output row-tile nt: [128, NO*128] where
    # partition p, col ch*128+n  =  X[nt*128+n, ch*128+p].
    xf = nc.dram_tensor("xf", [NPAIR, P, 2 * D], F16, kind="ExternalInput")
    xtt = nc.dram_tensor("xtt", [NT, P, D], F16, kind="ExternalInput")
    wf = nc.dram_tensor("wf", [D, D], F16, kind="ExternalInput")
    o_out = nc.dram_tensor("o_out", [N, SL], F16, kind="ExternalOutput")

    with tile.TileContext(nc) as tc:
        with (
            tc.tile_pool(name="big", bufs=1) as big,
            tc.tile_pool(name="xring", bufs=XRING) as xring,
            tc.tile_pool(name="tring", bufs=TRING) as tring,
            tc.tile_pool(name="stage", bufs=8) as stage,
            tc.tile_pool(name="psum", bufs=8, space="PSUM") as psum,
        ):
            Wsb = big.tile([P, NO, D], F16, tag="W")        # W   [e, a], 2MB
            Ssl = big.tile([P, NO, SL], F16, tag="Ssl")     # S_sl [e, d_sl]
            Msl = big.tile([P, NO, SL], F16, tag="Msl")     # M_sl [a, d_sl]

            # ---- S_sl = X^T X[:, sl], chunk-outer over the full batch
            accs = [
                psum.tile([P, 512], F32, tag="acc", name=f"sacc_{et}")[:, :SL]
                for et in range(NO)
            ]

            # PE warmup: junk matmuls on a zeroed scratch keep PE busy from
            # t~0 so the HAM clock-gate opens at ~3.4us.  Results land in
            # accs[0]'s bank and are wiped by its start=True at chunk 0.
            wz = stage.tile([P, P], F16, tag="warm")
            nc.vector.memset(wz[:], 0.0)
            for _ in range(52):
                nc.tensor.matmul(accs[0][:, :P], wz[:], wz[:], start=True, stop=True)

            gate_pair = None
            for pr in range(NPAIR):
                xc = xring.tile([P, 2 * D], F16, tag="xc")
                eng = nc.sync if pr % 2 == 0 else nc.scalar
                eng.dma_start(xc[:], xf[pr, :, :])
                if pr == GATE:
                    gate_pair = xc
                for half in range(2):
                    ch = 2 * pr + half
                    for et in range(NO):
                        nc.tensor.matmul(
                            accs[et][:],
                            xc[:, half * D + et * P : half * D + (et + 1) * P],
                            xc[:, half * D : half * D + SL],
                            start=(ch == 0),
                            stop=(ch == NCH - 1),
                        )

            # W rides both rings behind the xf stream: piece ch arrives
            # ~34+1.1*ch us, just ahead of the M phase's ch-outer loop.
            for ch in range(NO):
                eng = nc.sync if ch % 2 == 0 else nc.scalar
                eng.dma_start(Wsb[:, ch, :], wf[ch * P : (ch + 1) * P, :])

            # X^T tiles: each tile's DMA is WAW-gated behind a 1-column
            # dummy copy that reads xf pair GATE, so no X^T bytes move
            # before the S stream is nearly done; tiles 16+ ring-throttle
            # behind out-phase consumption.  Queues: evens sync (behind xf
            # evens), odds gpsimd (its only traffic).
            xts = []
            for nt in range(TRING):
                xt = tring.tile([P, D], F16, tag="xt", name=f"xt_{nt}")
                nc.vector.tensor_copy(xt[:, :1], gate_pair[:, :1])
                eng = nc.sync if nt % 2 == 0 else nc.gpsimd
                eng.dma_start(xt[:], xtt[nt, :, :])
                xts.append(xt)

            for et in range(NO):
                if et % 2 == 0:
                    nc.vector.tensor_copy(Ssl[:, et, :], accs[et][:])
                else:
                    nc.scalar.copy(Ssl[:, et, :], accs[et][:])

            # ---- M_sl = W^T S_sl, at-outer: each at's drain overlaps the
            # next at's matmuls, so Msl is fully staged when the out phase
            # begins (W is fully resident by M start).
            for at in range(NO):
                acc = psum.tile([P, 512], F32, tag="acc", name=f"macc_{at}")[:, :SL]
                for ch in range(NO):
                    nc.tensor.matmul(
                        acc[:],
                        Wsb[:, ch, at * P : (at + 1) * P],
                        Ssl[:, ch, :],
                        start=(ch == 0),
                        stop=(ch == NO - 1),
                    )
                if at % 2 == 0:
                    nc.vector.tensor_copy(Msl[:, at, :], acc[:])
                else:
                    nc.scalar.copy(Msl[:, at, :], acc[:])

            # ---- out[:, sl] = X M_sl : lhsT = X^T tile blocks, rhs = M_sl
            # X^T tile nt+TRING's DMA is issued right after group nt's MMs
            # (its ring-WAR sem is already posted, so no engine stall).
            for nt in range(NT):
                acc = psum.tile([P, 512], F32, tag="acc", name=f"oacc_{nt}")[:, :SL]
                for ch in range(NO):
                    nc.tensor.matmul(
                        acc[:],
                        xts[nt][:, ch * P : (ch + 1) * P],
                        Msl[:, ch, :],
                        start=(ch == 0),
                        stop=(ch == NO - 1),
                    )
                if nt + TRING < NT:
                    j = nt + TRING
                    xt = tring.tile([P, D], F16, tag="xt", name=f"xt_{j}")
                    eng = nc.sync if j % 2 == 0 else nc.gpsimd
                    eng.dma_start(xt[:], xtt[j, :, :])
                    xts.append(xt)
                ot = stage.tile([P, SL], F16, tag="ot")
                if nt % 2 == 0:
                    nc.vector.tensor_copy(ot[:], acc[:])
                else:
                    nc.scalar.copy(ot[:], acc[:])
                weng = nc.sync if nt % 2 == 0 else nc.gpsimd
                weng.dma_start(o_out[nt * P : (nt + 1) * P, :], ot[:])

    nc.finalize()
    return nc


def _get_compiled():
    global _compiled
    if _compiled is None:
        _compiled = _build()
    return _compiled


def kernel(hidden_states, queries, _trace=False, _trace_cores=None):
    x = np.ascontiguousarray(np.asarray(hidden_states, dtype=np.float32))
    w = np.ascontiguousarray(np.asarray(queries, dtype=np.float32))
    assert x.shape == (B, N, D) and w.shape == (D, D)

    nc = _get_compiled()
    w16 = w.astype(np.float16)
    x16 = [x[b].astype(np.float16) for b in range(B)]
    # X^T pre-tiled: xtt[nt, p, ch*128+n] = X[nt*128+n, ch*128+p]
    xtt16 = [
        np.ascontiguousarray(
            x16[b].T.reshape(NO, P, NT, P).transpose(2, 1, 0, 3).reshape(NT, P, D)
        )
        for b in range(B)
    ]
    in_maps = []
    for c in range(NCORES):
        b, s = c // GROUP, c % GROUP
        in_maps.append(
            {
                "xf": np.ascontiguousarray(np.roll(x16[b], -s * SL, axis=1)).reshape(NPAIR, P, 2 * D),
                "xtt": xtt16[b],
                "wf": np.ascontiguousarray(np.roll(w16, -s * SL, axis=0)),
            }
        )

    res = run_bass_kernel_spmd(
        nc,
        in_maps,
        core_ids=list(range(NCORES)),
        trace=_trace,
        trace_cores=_trace_cores,
    )

    out = np.empty((B, N, D), dtype=np.float32)
    for c in range(NCORES):
        b, s = c // GROUP, c % GROUP
        out[b, :, s * SL : (s + 1) * SL] = res.results[c]["o_out"].astype(np.float32)

    if _trace:
        kernel.last_result = res
    return out


# revision 16
# speedup vs baseline: 1.0258x; 1.0258x over previous
"""Trainium2 Bass kernel for nn_DenseAttentionOneHead — collective-free.

out_b = X_b (W^T (X_b^T X_b)).  The D (=1024) output columns split into 8
independent 256-column slices (4 per batch): per core,
  S_sl = X_b^T X_b[:, sl]     ([1024, 256], full-batch contraction)
  M_sl = W^T S_sl             ([1024, 256])
  out[:, sl] = X_b M_sl       ([4096, 256])

v5: xf streams in 512KB chunk-pairs at full HBM bandwidth; every X^T
tile DMA (xtt, host pre-tiled per output row-tile) is gated behind the
arrival of xf pair 12 via a 1-column dummy copy (so the Tile scheduler
cannot hoist X^T traffic into the S window), with a 16-deep ring giving
the out phase deep lookahead.  W rides the scalar ring between the
xf odds and the M phase.  Output is fp16 (host upcasts) written on the
sync ring behind the X^T evens.  A junk-matmul warmup opens the HAM
clock gate early.
"""

import numpy as np

import concourse.mybir as mybir
import concourse.tile as tile
from concourse import bacc
from concourse.bass_utils import run_bass_kernel_spmd

F32 = mybir.dt.float32
F16 = mybir.dt.float16
P = 128
D = 1024
B = 2
N = 4096
NCORES = 8
GROUP = 4            # cores per batch
SL = D // GROUP      # 256-column slice per core
NO = D // P          # 8 contraction chunks of D
NCH = N // P         # 32 row chunks of the full batch
NT = N // P          # 32 output row tiles
NPAIR = NCH // 2     # 16 xf 512KB chunk-pairs
XRING = 16           # all 16 xf pairs resident (8MB): DMAs free-run
TRING = 16           # X^T tile ring depth (4MB)
GATE = 12            # xf pair whose arrival releases the X^T stream

_compiled = None


def _build():
    nc = bacc.Bacc(None, target_bir_lowering=False, debug=False, num_devices=NCORES)

    # xf arrives column-rotated per core (its 256 target columns first) and
    # wf row-rotated identically, so the same program computes every slice.
    # xtt[nt] is X^T pre-tiled for output row-tile nt: [128, NO*128] where
    # partition p, col ch*128+n  =  X[nt*128+n, ch*128+p].
    xf = nc.dram_tensor("xf", [NPAIR, P, 2 * D], F16, kind="ExternalInput")
    xtt = nc.dram_tensor("xtt", [NT, P, D], F16, kind="ExternalInput")
    wf = nc.dram_tensor("wf", [D, D], F16, kind="ExternalInput")
    o_out = nc.dram_tensor("o_out", [N, SL], F16, kind="ExternalOutput")

    with tile.TileContext(nc) as tc:
        with (
            tc.tile_pool(name="big", bufs=1) as big,
            tc.tile_pool(name="xring", bufs=XRING) as xring,
            tc.tile_pool(name="tring", bufs=TRING) as tring,
            tc.tile_pool(name="stage", bufs=8) as stage,
            tc.tile_pool(name="psum", bufs=8, space="PSUM") as psum,
        ):
            Wsb = big.tile([P, NO, D], F16, tag="W")        # W   [e, a], 2MB
            Ssl = big.tile([P, NO, SL], F16, tag="Ssl")     # S_sl [e, d_sl]
            Msl = big.tile([P, NO, SL], F16, tag="Msl")     # M_sl [a, d_sl]

            # ---- S_sl = X^T X[:, sl], chunk-outer over the full batch
            accs = [
                psum.tile([P, 512], F32, tag="acc", name=f"sacc_{et}")[:, :SL]
                for et in range(NO)
            ]

            # PE warmup: junk matmuls on a zeroed scratch keep PE busy from
            # t~0 so the HAM clock-gate opens at ~3.4us.  Results land in
            # accs[0]'s bank and are wiped by its start=True at chunk 0.
            wz = stage.tile([P, P], F16, tag="warm")
            nc.vector.memset(wz[:], 0.0)
            for _ in range(52):
                nc.tensor.matmul(accs[0][:, :P], wz[:], wz[:], start=True, stop=True)

            gate_pair = None
            for pr in range(NPAIR):
                xc = xring.tile([P, 2 * D], F16, tag="xc")
                eng = nc.sync if pr % 2 == 0 else nc.scalar
                eng.dma_start(xc[:], xf[pr, :, :])
                if pr == GATE:
                    gate_pair = xc
                for half in range(2):
                    ch = 2 * pr + half
                    for et in range(NO):
                        nc.tensor.matmul(
                            accs[et][:],
                            xc[:, half * D + et * P : half * D + (et + 1) * P],
                            xc[:, half * D : half * D + SL],
                            start=(ch == 0),
                            stop=(ch == NCH - 1),
                        )

            # W rides both rings behind the xf stream: piece ch arrives
            # ~34+1.1*ch us, just ahead of the M phase's ch-outer loop.
            for ch in range(NO):
                eng = nc.sync if ch % 2 == 0 else nc.scalar
                eng.dma_start(Wsb[:, ch, :], wf[ch * P : (ch + 1) * P, :])

            # X^T tiles: each tile's DMA is WAW-gated behind a 1-column
            # dummy copy that reads xf pair GATE, so no X^T bytes move
            # before the S stream is nearly done; tiles 16+ ring-throttle
            # behind out-phase consumption.  Queues: evens sync (behind xf
            # evens), odds gpsimd (its only traffic).
            xts = []
            for nt in range(TRING):
                xt = tring.tile([P, D], F16, tag="xt", name=f"xt_{nt}")
                nc.vector.tensor_copy(xt[:, :1], gate_pair[:, :1])
                eng = nc.sync if nt % 2 == 0 else nc.gpsimd
                eng.dma_start(xt[:], xtt[nt, :, :])
                xts.append(xt)

            for et in range(NO):
                if et % 2 == 0:
                    nc.vector.tensor_copy(Ssl[:, et, :], accs[et][:])
                else:
                    nc.scalar.copy(Ssl[:, et, :], accs[et][:])

            # ---- M_sl = W^T S_sl, ch-outer: all 8 accumulators live; the
            # first MMs only wait on Ssl chunk drains already in flight.
            maccs = [
                psum.tile([P, 512], F32, tag="acc", name=f"macc_{at}")[:, :SL]
                for at in range(NO)
            ]
            for ch in range(NO):
                for at in range(NO):
                    nc.tensor.matmul(
                        maccs[at][:],
                        Wsb[:, ch, at * P : (at + 1) * P],
                        Ssl[:, ch, :],
                        start=(ch == 0),
                        stop=(ch == NO - 1),
                    )
            for at in range(NO):
                if at % 2 == 0:
                    nc.vector.tensor_copy(Msl[:, at, :], maccs[at][:])
                else:
                    nc.scalar.copy(Msl[:, at, :], maccs[at][:])

            # ---- out[:, sl] = X M_sl : lhsT = X^T tile blocks, rhs = M_sl
            # X^T tile nt+TRING's DMA is issued right after group nt's MMs
            # (its ring-WAR sem is already posted, so no engine stall).
            for nt in range(NT):
                acc = psum.tile([P, 512], F32, tag="acc", name=f"oacc_{nt}")[:, :SL]
                for ch in range(NO):
                    nc.tensor.matmul(
                        acc[:],
                        xts[nt][:, ch * P : (ch + 1) * P],
                        Msl[:, ch, :],
                        start=(ch == 0),
                        stop=(ch == NO - 1),
                    )
                if nt + TRING < NT:
                    j = nt + TRING
                    xt = tring.tile([P, D], F16, tag="xt", name=f"xt_{j}")
                    eng = nc.sync if j % 2 == 0 else nc.gpsimd
                    eng.dma_start(xt[:], xtt[j, :, :])
                    xts.append(xt)
                ot = stage.tile([P, SL], F16, tag="ot")
                if nt % 2 == 0:
                    nc.vector.tensor_copy(ot[:], acc[:])
                else:
                    nc.scalar.copy(ot[:], acc[:])
                weng = nc.sync if nt % 2 == 0 else nc.gpsimd
                weng.dma_start(o_out[nt * P : (nt + 1) * P, :], ot[:])

    nc.finalize()
    return nc


def _get_compiled():
    global _compiled
    if _compiled is None:
        _compiled = _build()
    return _compiled


def kernel(hidden_states, queries, _trace=False, _trace_cores=None):
    x = np.ascontiguousarray(np.asarray(hidden_states, dtype=np.float32))
    w = np.ascontiguousarray(np.asarray(queries, dtype=np.float32))
    assert x.shape == (B, N, D) and w.shape == (D, D)

    nc = _get_compiled()
    w16 = w.astype(np.float16)
    x16 = [x[b].astype(np.float16) for b in range(B)]
    # X^T pre-tiled: xtt[nt, p, ch*128+n] = X[nt*128+n, ch*128+p]
    xtt16 = [
        np.ascontiguousarray(
            x16[b].T.reshape(NO, P, NT, P).transpose(2, 1, 0, 3).reshape(NT, P, D)
        )
        for b in range(B)
    ]
    in_maps = []
    for c in range(NCORES):
        b, s = c // GROUP, c % GROUP
        in_maps.append(
            {
                "xf": np.ascontiguousarray(np.roll(x16[b], -s * SL, axis=1)).reshape(NPAIR, P, 2 * D),
                "xtt": xtt16[b],
                "wf": np.ascontiguousarray(np.roll(w16, -s * SL, axis=0)),
            }
        )

    res = run_bass_kernel_spmd(
        nc,
        in_maps,
        core_ids=list(range(NCORES)),
        trace=_trace,
        trace_cores=_trace_cores,
    )

    out = np.empty((B, N, D), dtype=np.float32)
    for c in range(NCORES):
        b, s = c // GROUP, c % GROUP
        out[b, :, s * SL : (s + 1) * SL] = res.results[c]["o_out"].astype(np.float32)

    if _trace:
        kernel.last_result = res
    return out


# revision 17
# speedup vs baseline: 1.0832x; 1.0560x over previous
"""Trainium2 Bass kernel for nn_DenseAttentionOneHead — collective-free.

out_b = X_b (W^T (X_b^T X_b)).  The D (=1024) output columns split into 8
independent 256-column slices (4 per batch): per core,
  S_sl = X_b^T X_b[:, sl]     ([1024, 256], full-batch contraction)
  M_sl = W^T S_sl             ([1024, 256])
  out[:, sl] = X_b M_sl       ([4096, 256])

v5: xf streams in 512KB chunk-pairs at full HBM bandwidth; every X^T
tile DMA (xtt, host pre-tiled per output row-tile) is gated behind the
arrival of xf pair 12 via a 1-column dummy copy (so the Tile scheduler
cannot hoist X^T traffic into the S window), with a 16-deep ring giving
the out phase deep lookahead.  W rides the scalar ring between the
xf odds and the M phase.  Output is fp16 (host upcasts) written on the
sync ring behind the X^T evens.  A junk-matmul warmup opens the HAM
clock gate early.
"""

import numpy as np

import concourse.mybir as mybir
import concourse.tile as tile
from concourse import bacc
from concourse.bass_utils import run_bass_kernel_spmd

F32 = mybir.dt.float32
F16 = mybir.dt.float16
P = 128
D = 1024
B = 2
N = 4096
NCORES = 8
GROUP = 4            # cores per batch
SL = D // GROUP      # 256-column slice per core
NO = D // P          # 8 contraction chunks of D
NCH = N // P         # 32 row chunks of the full batch
NT = N // P          # 32 output row tiles
NPAIR = NCH // 2     # 16 xf 512KB chunk-pairs
XRING = 16           # all 16 xf pairs resident (8MB): DMAs free-run
TRING = 16           # X^T tile ring depth (4MB)
GATE = 12            # xf pair whose arrival releases the X^T stream

_compiled = None


def _build():
    nc = bacc.Bacc(None, target_bir_lowering=False, debug=False, num_devices=NCORES)

    # xf arrives column-rotated per core (its 256 target columns first) and
    # wf row-rotated identically, so the same program computes every slice.
    # xtt[nt] is X^T pre-tiled for output row-tile nt: [128, NO*128] where
    # partition p, col ch*128+n  =  X[nt*128+n, ch*128+p].
    xf = nc.dram_tensor("xf", [NPAIR, P, 2 * D], F16, kind="ExternalInput")
    xtt = nc.dram_tensor("xtt", [NT, P, D], F16, kind="ExternalInput")
    wf = nc.dram_tensor("wf", [D, D], F16, kind="ExternalInput")
    o_out = nc.dram_tensor("o_out", [N, SL], F16, kind="ExternalOutput")

    with tile.TileContext(nc) as tc:
        with (
            tc.tile_pool(name="big", bufs=1) as big,
            tc.tile_pool(name="xring", bufs=XRING) as xring,
            tc.tile_pool(name="tring", bufs=TRING) as tring,
            tc.tile_pool(name="stage", bufs=8) as stage,
            tc.tile_pool(name="psum", bufs=8, space="PSUM") as psum,
        ):
            Wsb = big.tile([P, NO, D], F16, tag="W")        # W   [e, a], 2MB
            Ssl = big.tile([P, NO, SL], F16, tag="Ssl")     # S_sl [e, d_sl]
            Msl = big.tile([P, NO, SL], F16, tag="Msl")     # M_sl [a, d_sl]

            # ---- S_sl = X^T X[:, sl], chunk-outer over the full batch
            accs = [
                psum.tile([P, 512], F32, tag="acc", name=f"sacc_{et}")[:, :SL]
                for et in range(NO)
            ]

            # PE warmup: junk matmuls on a zeroed scratch keep PE busy from
            # t~0 so the HAM clock-gate opens at ~3.4us.  Results land in
            # accs[0]'s bank and are wiped by its start=True at chunk 0.
            wz = stage.tile([P, P], F16, tag="warm")
            nc.vector.memset(wz[:], 0.0)
            for _ in range(56):
                nc.tensor.matmul(accs[0][:, :P], wz[:], wz[:], start=True, stop=True)

            # Consumption order leads with sync-ring (even) pairs: the
            # scalar ring's first transfer starts ~2.4us later and its
            # completion sem lags ~3us more, so odd pairs get two extra
            # slots of grace.  Accumulation order is irrelevant to the sum.
            PAIR_ORDER = [0, 2, 1, 4, 3, 6, 5, 8, 7, 10, 9, 12, 11, 14, 13, 15]
            gate_pair = None
            xcs = {}
            for pr in range(NPAIR):
                xcs[pr] = xring.tile([P, 2 * D], F16, tag="xc", name=f"xc_{pr}")
                eng = nc.sync if pr % 2 == 0 else nc.scalar
                eng.dma_start(xcs[pr][:], xf[pr, :, :])
                if pr == GATE:
                    gate_pair = xcs[pr]
            for slot, pr in enumerate(PAIR_ORDER):
                xc = xcs[pr]
                for half in range(2):
                    for et in range(NO):
                        nc.tensor.matmul(
                            accs[et][:],
                            xc[:, half * D + et * P : half * D + (et + 1) * P],
                            xc[:, half * D : half * D + SL],
                            start=(slot == 0 and half == 0),
                            stop=(slot == NPAIR - 1 and half == 1),
                        )

            # W rides both rings behind the xf stream: piece ch arrives
            # ~34+1.1*ch us, just ahead of the M phase's ch-outer loop.
            for ch in range(NO):
                eng = nc.sync if ch % 2 == 0 else nc.scalar
                eng.dma_start(Wsb[:, ch, :], wf[ch * P : (ch + 1) * P, :])

            # X^T tiles: each tile's DMA is WAW-gated behind a 1-column
            # dummy copy that reads xf pair GATE, so no X^T bytes move
            # before the S stream is nearly done; tiles 16+ ring-throttle
            # behind out-phase consumption.  Queues: evens sync (behind xf
            # evens), odds gpsimd (its only traffic).
            xts = []
            for nt in range(TRING):
                xt = tring.tile([P, D], F16, tag="xt", name=f"xt_{nt}")
                nc.vector.tensor_copy(xt[:, :1], gate_pair[:, :1])
                eng = nc.sync if nt % 2 == 0 else nc.gpsimd
                eng.dma_start(xt[:], xtt[nt, :, :])
                xts.append(xt)

            for et in range(NO):
                if et % 2 == 0:
                    nc.vector.tensor_copy(Ssl[:, et, :], accs[et][:])
                else:
                    nc.scalar.copy(Ssl[:, et, :], accs[et][:])

            # ---- M_sl = W^T S_sl, ch-outer: all 8 accumulators live; the
            # first MMs only wait on Ssl chunk drains already in flight.
            maccs = [
                psum.tile([P, 512], F32, tag="acc", name=f"macc_{at}")[:, :SL]
                for at in range(NO)
            ]
            for ch in range(NO):
                for at in range(NO):
                    nc.tensor.matmul(
                        maccs[at][:],
                        Wsb[:, ch, at * P : (at + 1) * P],
                        Ssl[:, ch, :],
                        start=(ch == 0),
                        stop=(ch == NO - 1),
                    )
            for at in range(NO):
                if at % 2 == 0:
                    nc.vector.tensor_copy(Msl[:, at, :], maccs[at][:])
                else:
                    nc.scalar.copy(Msl[:, at, :], maccs[at][:])

            # ---- out[:, sl] = X M_sl : lhsT = X^T tile blocks, rhs = M_sl
            # X^T tile nt+TRING's DMA is issued right after group nt's MMs
            # (its ring-WAR sem is already posted, so no engine stall).
            for nt in range(NT):
                acc = psum.tile([P, 512], F32, tag="acc", name=f"oacc_{nt}")[:, :SL]
                for ch in range(NO):
                    nc.tensor.matmul(
                        acc[:],
                        xts[nt][:, ch * P : (ch + 1) * P],
                        Msl[:, ch, :],
                        start=(ch == 0),
                        stop=(ch == NO - 1),
                    )
                if nt + TRING < NT:
                    j = nt + TRING
                    xt = tring.tile([P, D], F16, tag="xt", name=f"xt_{j}")
                    eng = nc.sync if j % 2 == 0 else nc.gpsimd
                    eng.dma_start(xt[:], xtt[j, :, :])
                    xts.append(xt)
                ot = stage.tile([P, SL], F16, tag="ot")
                if nt % 2 == 0:
                    nc.vector.tensor_copy(ot[:], acc[:])
                else:
                    nc.scalar.copy(ot[:], acc[:])
                weng = nc.sync if nt % 2 == 0 else nc.scalar
                weng.dma_start(o_out[nt * P : (nt + 1) * P, :], ot[:])

    nc.finalize()
    return nc


def _get_compiled():
    global _compiled
    if _compiled is None:
        _compiled = _build()
    return _compiled


def kernel(hidden_states, queries, _trace=False, _trace_cores=None):
    x = np.ascontiguousarray(np.asarray(hidden_states, dtype=np.float32))
    w = np.ascontiguousarray(np.asarray(queries, dtype=np.float32))
    assert x.shape == (B, N, D) and w.shape == (D, D)

    nc = _get_compiled()
    w16 = w.astype(np.float16)
    x16 = [x[b].astype(np.float16) for b in range(B)]
    # X^T pre-tiled: xtt[nt, p, ch*128+n] = X[nt*128+n, ch*128+p]
    xtt16 = [
        np.ascontiguousarray(
            x16[b].T.reshape(NO, P, NT, P).transpose(2, 1, 0, 3).reshape(NT, P, D)
        )
        for b in range(B)
    ]
    in_maps = []
    for c in range(NCORES):
        b, s = c // GROUP, c % GROUP
        in_maps.append(
            {
                "xf": np.ascontiguousarray(np.roll(x16[b], -s * SL, axis=1)).reshape(NPAIR, P, 2 * D),
                "xtt": xtt16[b],
                "wf": np.ascontiguousarray(np.roll(w16, -s * SL, axis=0)),
            }
        )

    res = run_bass_kernel_spmd(
        nc,
        in_maps,
        core_ids=list(range(NCORES)),
        trace=_trace,
        trace_cores=_trace_cores,
    )

    out = np.empty((B, N, D), dtype=np.float32)
    for c in range(NCORES):
        b, s = c // GROUP, c % GROUP
        out[b, :, s * SL : (s + 1) * SL] = res.results[c]["o_out"].astype(np.float32)

    if _trace:
        kernel.last_result = res
    return out
